# revision 24
# baseline (speedup 1.0000x reference)
"""N-gram embedding lookup kernel for Trainium2 (8 NeuronCores, Bass/Tile).

Problem: for each token x[b,s] (vocab 50000), gather precomputed n-gram
hash ids for orders 1..3 (12+11+10 slots), gather embedding rows from
three tables (1001/10001/50001 x 256 fp32), masked-mean each order,
concat to 768 dims; tokens x<4 take tab0[x] instead.

Environment constraints (verified on HW this session):
 - no HIPI ucode => custom bulk-gather (InstDMAGatherAnt) is unavailable
   (NRT_EXEC_UNIT_UNRECOVERABLE when executed);
 - walrus-native indirect DMA (InstDMACopy + dynamic AP) gathers exactly
   ONE row per partition per instruction (extra offset-AP indices are
   ignored; the descriptor reads out-row-size contiguous bytes from the
   single per-partition index), and each instruction serializes ~1.7us
   (bypass) / ~2.2us (CCE-add) of Pool-engine SWDGE descriptor
   generation. That serial gen time dominates; num_swdge_queues,
   acc dtype, and chain splitting measurably do NOT change it, so the
   only lever is issuing FEWER indirect-DMA instructions.

Design (measured 338us/core vs ~970us for the naive 476-instruction
data-parallel layout; rel err 1.66e-3):
 - host: dedup x to unique words, sort by word length (cnt1 desc), deal
   round-robin to the 8 cores => every 128-token group holds words of
   nearly equal length, and per-group slot counts (the "profile")
   shrink from (12,11,10) to the group's actual max cnt per order. The
   Bass program is compiled per profile (cached) and outputs are
   scattered back to token positions on host.
 - host: per-token meta rows (ids/cnts, int32 [P, G*40]) are shipped
   directly; specials (x<4) are folded into the tables as 4 appended
   rows so no separate patch pass exists.
 - chip, order 1 (1001-row table): offloaded off the Pool engine
   entirely - DVE builds per-token bucket histograms via f16 iota
   is_equal one-hots, PE transposes them and matmuls against the
   SBUF-resident padded table, accumulating in PSUM.
 - chip, orders 2/3: profile[g][o] independent bypass indirect-DMA
   gathers (bf16 rows -> f32 cast in the DMA) into slot slabs, then one
   DVE tensor_reduce + 1/cnt scale per (group, order). Slabs rotate
   5-deep; each group's reduce is issued one group late so the Pool
   engine never waits on DVE.
 - per-group stores overlap the Pool gather stream on the SP engine.
"""
import numpy as np
import ml_dtypes
from contextlib import ExitStack

from concourse import bacc, bass, mybir, tile
from concourse.bass_utils import run_bass_kernel_spmd

BF16 = ml_dtypes.bfloat16

NCORES = 8
B, S = 8, 2048
TOK = B * S
TPC = TOK // NCORES          # 2048 tokens per core
P = 128
G = TPC // P                 # 16 groups
EMB = 256
V = 50000
LS = (12, 11, 10)
COLBASE = (0, 12, 23)        # meta col of slot 0 per order
CNTCOL = 33                  # meta cols 33..35 = cnt1..3
MW = 40                      # meta row width (int32)
NQ = 1                       # SWDGE queues to spray across
OFFLOAD1 = True              # compute order-1 via DVE one-hots + PE matmul
META_HOST = True             # ship per-token meta rows from host (no gather)
SLOTS = True                 # bypass gathers into slot slabs + DVE reduce
W1 = 1024                    # order-1 bucket space (1005 used, padded)
NB1 = W1 // 128


def _build(profile, num_swdge_queues=NQ, unroll=1, offload1=OFFLOAD1,
           meta_host=META_HOST, cce_off=False, acc_bf16=False, rot=0,
           slots=SLOTS):
    G = len(profile)
    TPC = G * P
    i32, f32, bf16 = mybir.dt.int32, mybir.dt.float32, mybir.dt.bfloat16
    f16 = mybir.dt.float16
    nc = bacc.Bacc("TRN2", target_bir_lowering=False, debug=False,
                   num_devices=NCORES, num_swdge_queues=num_swdge_queues)

    if meta_host:
        d_metaT = nc.dram_tensor("metaT", [P, G * MW], i32,
                                 kind="ExternalInput")
    else:
        d_meta = nc.dram_tensor("metaI", [V, MW], i32, kind="ExternalInput")
        d_xpg = nc.dram_tensor("xpg", [P, G], i32, kind="ExternalInput")
    d_tabs = [nc.dram_tensor(f"tab{o+1}z", [(1005, 10005, 50005)[o], EMB],
                             bf16, kind="ExternalInput") for o in range(3)]
    d_out = nc.dram_tensor("out", [TPC, 768], f32, kind="ExternalOutput")
    if offload1:
        d_iota = nc.dram_tensor("iotaH", [P, W1], f16, kind="ExternalInput")
        d_tab1p = nc.dram_tensor("tab1p", [W1, EMB], bf16,
                                 kind="ExternalInput")

    swdge_q = [0]

    def spray(inst):
        # round-robin independent DMAs across SWDGE queues; keep each
        # accumulation chain on one queue (callers rotate per chain)
        if num_swdge_queues > 1:
            q = swdge_q[0] % num_swdge_queues
            if q:
                inst.ins.queue = f"qPoolDynamic{q}"
        return inst

    def next_q():
        swdge_q[0] += 1

    with ExitStack() as ctx:
        tc = ctx.enter_context(tile.TileContext(nc))
        pool = ctx.enter_context(tc.tile_pool(name="sbuf", bufs=1))

        t_xpg = pool.tile([P, G], i32)
        t_meta = pool.tile([P, G * MW], i32)
        o_lo = 1 if offload1 else 0
        acc_dt = bf16 if acc_bf16 else f32
        rot_tiles = [pool.tile([P, EMB], acc_dt, name=f"rot_{i}")
                     for i in range(rot)] if rot else None
        if slots:
            lmax = {o: max(p[o] for p in profile) for o in range(o_lo, 3)}
            slabs = {o: [pool.tile([P, lmax[o] * EMB], f32,
                                   name=f"slab_{o}_{i}") for i in range(5)]
                     for o in range(o_lo, 3)}
            accs = None
        else:
            accs = {(g, o): (rot_tiles[(g * 3 + o) % rot] if rot else
                             pool.tile([P, EMB], acc_dt, name=f"acc_{g}_{o}"))
                    for g in range(G) for o in range(o_lo, 3)}
        t_red = pool.tile([P, EMB], f32, name='t_red') if slots else None
        t_cntf = pool.tile([P, G * 3], f32)
        t_rcp = pool.tile([P, G * 3], f32)
        t_out = pool.tile([P, G * 768], f32)
        if offload1:
            from concourse.masks import make_identity
            psum = ctx.enter_context(
                tc.tile_pool(name="psum", bufs=1, space="PSUM"))
            t_iota = pool.tile([P, W1], f16)
            t_tab1 = pool.tile([P, NB1 * EMB], bf16)
            t_id = pool.tile([P, P], f16)
            t_ids1f = pool.tile([P, G * 12], f16)
            t_H = [pool.tile([P, W1], f16, name=f"H{i}") for i in range(2)]
            t_oh = [pool.tile([P, W1], f16, name=f"oh{i}") for i in range(2)]
            t_HT = [pool.tile([P, NB1 * P], bf16, name=f"HT{i}")
                    for i in range(2)]
            ps_T = [psum.tile([P, P], f16, name=f"psT{i}") for i in range(2)]
            ps_E = [psum.tile([P, EMB], f32, name=f"psE{i}")
                    for i in range(4)]
            nc.sync.dma_start(out=t_iota[:], in_=d_iota[:])
            nc.sync.dma_start(
                out=bass.AP(t_tab1[:].tensor, 0,
                            [t_tab1[:].ap[0], [EMB, NB1], [1, EMB]]),
                in_=bass.AP(d_tab1p, 0, [[EMB, P], [P * EMB, NB1], [1, EMB]]))
            make_identity(nc, t_id[:])

        if unroll > 1:
            # hardware loop for benchmarking: body is idempotent
            ctx.enter_context(tc.For_i(0, unroll))

        if True:
            if meta_host:
                nc.sync.dma_start(out=t_meta[:], in_=d_metaT[:])
            else:
                nc.sync.dma_start(out=t_xpg[:], in_=d_xpg[:])
                # ---- meta gathers: one per group, row [40 int32] per token
                for g in range(G):
                    spray(nc.gpsimd.indirect_dma_start(
                        out=t_meta[:, g * MW:(g + 1) * MW],
                        out_offset=None,
                        in_=d_meta[:],
                        in_offset=bass.IndirectOffsetOnAxis(
                            ap=t_xpg[:, g:g + 1], axis=0)))
                    next_q()

            # ---- 1/cnt
            nc.vector.tensor_copy(
                out=t_cntf[:],
                in_=bass.AP(t_meta[:].tensor, CNTCOL,
                            [t_meta[:].ap[0], [MW, G], [1, 3]]))
            nc.vector.reciprocal(out=t_rcp[:], in_=t_cntf[:])

            # ---- per-group pipeline: Pool gathers / DVE+PE order-1 /
            #      DVE reduces+scales / SP store.  Issue order per group
            #      keeps slab/H/psum rotation windows correct and lets
            #      every engine run concurrently.
            rix = [0]

            def emit_tail(g):
                # DVE: reduce slots (v4) / scale into output tile
                for o in range(o_lo, 3):
                    out_ap = t_out[:, g * 768 + o * 256:
                                   g * 768 + (o + 1) * 256]
                    in1 = bass.AP(t_rcp[:].tensor, g * 3 + o,
                                  [t_rcp[:].ap[0], [0, 256]])
                    if slots:
                        L = profile[g][o]
                        slab = slabs[o][g % 5]
                        nc.vector.tensor_reduce(
                            out=out_ap,
                            in_=bass.AP(slab[:].tensor, 0,
                                        [slab[:].ap[0], [1, EMB], [EMB, L]]),
                            axis=mybir.AxisListType.X,
                            op=mybir.AluOpType.add, opt_input=False)
                        nc.vector.tensor_tensor(out=out_ap, in0=out_ap,
                                                in1=in1,
                                                op=mybir.AluOpType.mult)
                    else:
                        acc = accs[(g, o)]
                        nc.vector.tensor_tensor(out=out_ap, in0=acc[:],
                                                in1=in1,
                                                op=mybir.AluOpType.mult)
                # store: SBUF (p, g, 768) -> DRAM row g*128+p
                nc.sync.dma_start(
                    out=bass.AP(d_out, g * P * 768, [[768, P], [1, 768]]),
                    in_=t_out[:, g * 768:(g + 1) * 768])

            for g in range(G):
                # Pool: embedding gathers for orders 2..3
                for o in range(o_lo, 3):
                    L = profile[g][o]
                    for s in range(L):
                        col = g * MW + COLBASE[o] + s
                        if slots:
                            slab = slabs[o][g % 5]
                            dst = slab[:, s * EMB:(s + 1) * EMB]
                            op = mybir.AluOpType.bypass
                        elif rot:
                            dst = rot_tiles[rix[0] % rot][:]
                            rix[0] += 1
                            op = mybir.AluOpType.bypass
                        else:
                            dst = accs[(g, o)][:]
                            op = (mybir.AluOpType.bypass
                                  if (s == 0 or cce_off)
                                  else mybir.AluOpType.add)
                        spray(nc.gpsimd.indirect_dma_start(
                            out=dst,
                            out_offset=None,
                            in_=d_tabs[o][:],
                            in_offset=bass.IndirectOffsetOnAxis(
                                ap=t_meta[:, col:col + 1], axis=0),
                            compute_op=op))
                    next_q()

                if g > 0:
                    emit_tail(g - 1)

                if offload1:
                    # order-1 via one-hot histogram + PE matmul
                    s1 = profile[g][0]
                    H = t_H[g % 2]
                    # ids (int32 meta cols) -> f16, exact for ids <= 2048
                    nc.vector.tensor_copy(
                        out=t_ids1f[:, g * 12:g * 12 + s1],
                        in_=t_meta[:, g * MW:g * MW + s1])
                    for s in range(s1):
                        idb = bass.AP(t_ids1f[:].tensor, g * 12 + s,
                                      [t_ids1f[:].ap[0], [0, W1]])
                        if s == 0:
                            nc.vector.tensor_tensor(
                                out=H[:], in0=t_iota[:], in1=idb,
                                op=mybir.AluOpType.is_equal)
                        else:
                            oh = t_oh[g % 2]
                            nc.vector.tensor_tensor(
                                out=oh[:], in0=t_iota[:], in1=idb,
                                op=mybir.AluOpType.is_equal)
                            nc.vector.tensor_tensor(
                                out=H[:], in0=H[:], in1=oh[:],
                                op=mybir.AluOpType.add)
                    HT = t_HT[g % 2]
                    for k in range(NB1):
                        pT = ps_T[k % 2]
                        nc.tensor.transpose(
                            pT[:], H[:, k * P:(k + 1) * P], t_id[:])
                        nc.vector.tensor_copy(
                            out=HT[:, k * P:(k + 1) * P], in_=pT[:])
                    pE = ps_E[g % 4]
                    for k in range(NB1):
                        nc.tensor.matmul(
                            pE[:],
                            lhsT=HT[:, k * P:(k + 1) * P],
                            rhs=t_tab1[:, k * EMB:(k + 1) * EMB],
                            start=(k == 0), stop=(k == NB1 - 1))
                    in1 = bass.AP(t_rcp[:].tensor, g * 3 + 0,
                                  [t_rcp[:].ap[0], [0, EMB]])
                    nc.vector.tensor_tensor(
                        out=t_out[:, g * 768:g * 768 + EMB],
                        in0=pE[:], in1=in1, op=mybir.AluOpType.mult)

            emit_tail(G - 1)

    return nc


_NC_CACHE = {}


def _get_nc(profile, nq=NQ, unroll=1, offload1=OFFLOAD1,
            meta_host=META_HOST, **kw):
    key = (profile, nq, unroll, offload1, meta_host, tuple(sorted(kw.items())))
    if key not in _NC_CACHE:
        nc = _build(profile, num_swdge_queues=nq, unroll=unroll,
                    offload1=offload1, meta_host=meta_host, **kw)
        nc.finalize()
        _NC_CACHE[key] = nc
    return _NC_CACHE[key]


def _prep(inputs):
    tab0 = np.asarray(inputs['tab0'], np.float32)
    tabs = [np.asarray(inputs[f'tab{o+1}'], np.float32) for o in range(3)]
    ids = [np.asarray(inputs[f'ids{o+1}'], np.int64) for o in range(3)]
    cnt = [np.asarray(inputs[f'cnt{o+1}'], np.int64) for o in range(3)]

    meta = np.zeros((V, MW), np.int32)
    for o in range(3):
        meta[:, COLBASE[o]:COLBASE[o] + LS[o]] = ids[o]
        meta[:, CNTCOL + o] = cnt[o]
    # specials: slot 0 -> appended per-special row, others 0, cnt 1
    meta[:4, :CNTCOL] = 0
    meta[:4, CNTCOL:CNTCOL + 3] = 1
    nrows = (1001, 10001, 50001)
    for o in range(3):
        meta[:4, COLBASE[o]] = nrows[o] + np.arange(4)

    shared = {'metaI': meta}
    for o in range(3):
        tz = np.zeros((nrows[o] + 4, EMB), BF16)
        tz[1:nrows[o]] = tabs[o][1:].astype(BF16)
        tz[nrows[o]:] = tab0[:, o * EMB:(o + 1) * EMB].astype(BF16)
        shared[f'tab{o+1}z'] = tz
    shared['iotaH'] = np.tile(np.arange(W1, dtype=np.float16)[None, :],
                              (P, 1))
    tab1p = np.zeros((W1, EMB), BF16)
    tab1p[:1005] = shared['tab1z']
    shared['tab1p'] = tab1p

    # ---- dedup words, sort by length (cnt1 desc), deal to cores
    x = np.asarray(inputs['x'], np.int64).reshape(-1)
    ux, inv = np.unique(x, return_inverse=True)
    key = meta[ux, CNTCOL]                     # cnt1 = word length
    order_u = np.argsort(-key, kind='stable')  # descending
    su = ux[order_u]                           # sorted unique words
    n_u = len(su)
    n_pad = -(-n_u // (NCORES * P)) * (NCORES * P)
    su = np.concatenate([su, np.zeros(n_pad - n_u, np.int64)])
    Gc = n_pad // (NCORES * P)                 # groups per core

    core_words = [su[c::NCORES] for c in range(NCORES)]

    profile = []
    for g in range(Gc):
        mx = [1, 1, 1]
        for c in range(NCORES):
            seg = core_words[c][g * P:(g + 1) * P]
            for o in range(3):
                mx[o] = max(mx[o], int(meta[seg, CNTCOL + o].max()))
        profile.append(tuple(mx))
    profile = tuple(profile)

    # token t -> rank of its word in su -> (core r%8, row r//8)
    rank_of = np.empty(n_u, np.int64)
    rank_of[order_u] = np.arange(n_u)
    tok_rank = rank_of[inv]

    in_maps = []
    for c in range(NCORES):
        m = dict(shared)
        m['xpg'] = np.ascontiguousarray(
            core_words[c].reshape(Gc, P).T).astype(np.int32)
        m['metaT'] = np.ascontiguousarray(
            meta[core_words[c]].reshape(Gc, P, MW)
            .transpose(1, 0, 2).reshape(P, Gc * MW))
        in_maps.append(m)
    return in_maps, profile, tok_rank

def _run(nc, in_maps, trace=False):
    return run_bass_kernel_spmd(nc, in_maps, core_ids=list(range(NCORES)),
                                trace=trace)


def kernel(**inputs):
    in_maps, profile, tok_rank = _prep(inputs)
    nc = _get_nc(profile)
    res = _run(nc, in_maps)
    by_rank = np.stack([np.asarray(res.results[c]['out'])
                        for c in range(NCORES)])      # [core, row, 768]
    out = by_rank[tok_rank % NCORES, tok_rank // NCORES]
    return out.reshape(B, S, 768)


# revision 25
# speedup vs baseline: 1.0545x; 1.0545x over previous
"""N-gram embedding lookup kernel for Trainium2 (8 NeuronCores, Bass/Tile).

Problem: for each token x[b,s] (vocab 50000), gather precomputed n-gram
hash ids for orders 1..3 (12+11+10 slots), gather embedding rows from
three tables (1001/10001/50001 x 256 fp32), masked-mean each order,
concat to 768 dims; tokens x<4 take tab0[x] instead.

Environment constraints (verified on HW this session):
 - no HIPI ucode => custom bulk-gather (InstDMAGatherAnt) is unavailable
   (NRT_EXEC_UNIT_UNRECOVERABLE when executed);
 - walrus-native indirect DMA (InstDMACopy + dynamic AP) gathers exactly
   ONE row per partition per instruction (extra offset-AP indices are
   ignored; the descriptor reads out-row-size contiguous bytes from the
   single per-partition index), and each instruction serializes ~1.7us
   (bypass) / ~2.2us (CCE-add) of Pool-engine SWDGE descriptor
   generation. That serial gen time dominates; num_swdge_queues,
   acc dtype, and chain splitting measurably do NOT change it, so the
   only lever is issuing FEWER indirect-DMA instructions.

Design (measured 338us/core vs ~970us for the naive 476-instruction
data-parallel layout; rel err 1.66e-3):
 - host: dedup x to unique words, sort by word length (cnt1 desc), deal
   round-robin to the 8 cores => every 128-token group holds words of
   nearly equal length, and per-group slot counts (the "profile")
   shrink from (12,11,10) to the group's actual max cnt per order. The
   Bass program is compiled per profile (cached) and outputs are
   scattered back to token positions on host.
 - host: per-token meta rows (ids/cnts, int32 [P, G*40]) are shipped
   directly; specials (x<4) are folded into the tables as 4 appended
   rows so no separate patch pass exists.
 - chip, order 1 (1001-row table): offloaded off the Pool engine
   entirely - DVE builds per-token bucket histograms via f16 iota
   is_equal one-hots, PE transposes them and matmuls against the
   SBUF-resident padded table, accumulating in PSUM.
 - chip, orders 2/3: profile[g][o] independent bypass indirect-DMA
   gathers (bf16 rows -> f32 cast in the DMA) into slot slabs, then one
   DVE tensor_reduce + 1/cnt scale per (group, order). Slabs rotate
   5-deep; each group's reduce is issued two groups late so the Pool
   engine never waits on DVE (measured -18us vs one group late).
 - per-group stores overlap the Pool gather stream on the SP engine.
"""
import numpy as np
import ml_dtypes
from contextlib import ExitStack

from concourse import bacc, bass, mybir, tile
from concourse.bass_utils import run_bass_kernel_spmd

BF16 = ml_dtypes.bfloat16

NCORES = 8
B, S = 8, 2048
TOK = B * S
TPC = TOK // NCORES          # 2048 tokens per core
P = 128
G = TPC // P                 # 16 groups
EMB = 256
V = 50000
LS = (12, 11, 10)
COLBASE = (0, 12, 23)        # meta col of slot 0 per order
CNTCOL = 33                  # meta cols 33..35 = cnt1..3
MW = 40                      # meta row width (int32)
NQ = 1                       # SWDGE queues to spray across
OFFLOAD1 = True              # compute order-1 via DVE one-hots + PE matmul
META_HOST = True             # ship per-token meta rows from host (no gather)
SLOTS = True                 # bypass gathers into slot slabs + DVE reduce
W1 = 1024                    # order-1 bucket space (1005 used, padded)
NB1 = W1 // 128


def _build(profile, num_swdge_queues=NQ, unroll=1, offload1=OFFLOAD1,
           meta_host=META_HOST, cce_off=False, acc_bf16=False, rot=0,
           slots=SLOTS):
    G = len(profile)
    TPC = G * P
    i32, f32, bf16 = mybir.dt.int32, mybir.dt.float32, mybir.dt.bfloat16
    f16 = mybir.dt.float16
    nc = bacc.Bacc("TRN2", target_bir_lowering=False, debug=False,
                   num_devices=NCORES, num_swdge_queues=num_swdge_queues)

    if meta_host:
        d_metaT = nc.dram_tensor("metaT", [P, G * MW], i32,
                                 kind="ExternalInput")
    else:
        d_meta = nc.dram_tensor("metaI", [V, MW], i32, kind="ExternalInput")
        d_xpg = nc.dram_tensor("xpg", [P, G], i32, kind="ExternalInput")
    d_tabs = [nc.dram_tensor(f"tab{o+1}z", [(1005, 10005, 50005)[o], EMB],
                             bf16, kind="ExternalInput") for o in range(3)]
    d_out = nc.dram_tensor("out", [TPC, 768], f32, kind="ExternalOutput")
    if offload1:
        d_iota = nc.dram_tensor("iotaH", [P, W1], f16, kind="ExternalInput")
        d_tab1p = nc.dram_tensor("tab1p", [W1, EMB], bf16,
                                 kind="ExternalInput")

    swdge_q = [0]

    def spray(inst):
        # round-robin independent DMAs across SWDGE queues; keep each
        # accumulation chain on one queue (callers rotate per chain)
        if num_swdge_queues > 1:
            q = swdge_q[0] % num_swdge_queues
            if q:
                inst.ins.queue = f"qPoolDynamic{q}"
        return inst

    def next_q():
        swdge_q[0] += 1

    with ExitStack() as ctx:
        tc = ctx.enter_context(tile.TileContext(nc))
        pool = ctx.enter_context(tc.tile_pool(name="sbuf", bufs=1))

        t_xpg = pool.tile([P, G], i32)
        t_meta = pool.tile([P, G * MW], i32)
        o_lo = 1 if offload1 else 0
        acc_dt = bf16 if acc_bf16 else f32
        rot_tiles = [pool.tile([P, EMB], acc_dt, name=f"rot_{i}")
                     for i in range(rot)] if rot else None
        if slots:
            lmax = {o: max(p[o] for p in profile) for o in range(o_lo, 3)}
            slabs = {o: [pool.tile([P, lmax[o] * EMB], f32,
                                   name=f"slab_{o}_{i}") for i in range(5)]
                     for o in range(o_lo, 3)}
            accs = None
        else:
            accs = {(g, o): (rot_tiles[(g * 3 + o) % rot] if rot else
                             pool.tile([P, EMB], acc_dt, name=f"acc_{g}_{o}"))
                    for g in range(G) for o in range(o_lo, 3)}
        t_red = pool.tile([P, EMB], f32, name='t_red') if slots else None
        t_cntf = pool.tile([P, G * 3], f32)
        t_rcp = pool.tile([P, G * 3], f32)
        t_out = pool.tile([P, G * 768], f32)
        if offload1:
            from concourse.masks import make_identity
            psum = ctx.enter_context(
                tc.tile_pool(name="psum", bufs=1, space="PSUM"))
            t_iota = pool.tile([P, W1], f16)
            t_tab1 = pool.tile([P, NB1 * EMB], bf16)
            t_id = pool.tile([P, P], f16)
            t_ids1f = pool.tile([P, G * 12], f16)
            t_H = [pool.tile([P, W1], f16, name=f"H{i}") for i in range(2)]
            t_oh = [pool.tile([P, W1], f16, name=f"oh{i}") for i in range(2)]
            t_HT = [pool.tile([P, NB1 * P], bf16, name=f"HT{i}")
                    for i in range(2)]
            ps_T = [psum.tile([P, P], f16, name=f"psT{i}") for i in range(2)]
            ps_E = [psum.tile([P, EMB], f32, name=f"psE{i}")
                    for i in range(4)]
            nc.sync.dma_start(out=t_iota[:], in_=d_iota[:])
            nc.sync.dma_start(
                out=bass.AP(t_tab1[:].tensor, 0,
                            [t_tab1[:].ap[0], [EMB, NB1], [1, EMB]]),
                in_=bass.AP(d_tab1p, 0, [[EMB, P], [P * EMB, NB1], [1, EMB]]))
            make_identity(nc, t_id[:])

        if unroll > 1:
            # hardware loop for benchmarking: body is idempotent
            ctx.enter_context(tc.For_i(0, unroll))

        if True:
            if meta_host:
                nc.sync.dma_start(out=t_meta[:], in_=d_metaT[:])
            else:
                nc.sync.dma_start(out=t_xpg[:], in_=d_xpg[:])
                # ---- meta gathers: one per group, row [40 int32] per token
                for g in range(G):
                    spray(nc.gpsimd.indirect_dma_start(
                        out=t_meta[:, g * MW:(g + 1) * MW],
                        out_offset=None,
                        in_=d_meta[:],
                        in_offset=bass.IndirectOffsetOnAxis(
                            ap=t_xpg[:, g:g + 1], axis=0)))
                    next_q()

            # ---- 1/cnt
            nc.vector.tensor_copy(
                out=t_cntf[:],
                in_=bass.AP(t_meta[:].tensor, CNTCOL,
                            [t_meta[:].ap[0], [MW, G], [1, 3]]))
            nc.vector.reciprocal(out=t_rcp[:], in_=t_cntf[:])

            # ---- per-group pipeline: Pool gathers / DVE+PE order-1 /
            #      DVE reduces+scales / SP store.  Issue order per group
            #      keeps slab/H/psum rotation windows correct and lets
            #      every engine run concurrently.
            rix = [0]

            def emit_tail(g):
                # DVE: reduce slots (v4) / scale into output tile
                for o in range(o_lo, 3):
                    out_ap = t_out[:, g * 768 + o * 256:
                                   g * 768 + (o + 1) * 256]
                    in1 = bass.AP(t_rcp[:].tensor, g * 3 + o,
                                  [t_rcp[:].ap[0], [0, 256]])
                    if slots:
                        L = profile[g][o]
                        slab = slabs[o][g % 5]
                        nc.vector.tensor_reduce(
                            out=out_ap,
                            in_=bass.AP(slab[:].tensor, 0,
                                        [slab[:].ap[0], [1, EMB], [EMB, L]]),
                            axis=mybir.AxisListType.X,
                            op=mybir.AluOpType.add, opt_input=False)
                        nc.vector.tensor_tensor(out=out_ap, in0=out_ap,
                                                in1=in1,
                                                op=mybir.AluOpType.mult)
                    else:
                        acc = accs[(g, o)]
                        nc.vector.tensor_tensor(out=out_ap, in0=acc[:],
                                                in1=in1,
                                                op=mybir.AluOpType.mult)
                # store: SBUF (p, g, 768) -> DRAM row g*128+p
                nc.sync.dma_start(
                    out=bass.AP(d_out, g * P * 768, [[768, P], [1, 768]]),
                    in_=t_out[:, g * 768:(g + 1) * 768])

            for g in range(G):
                # Pool: embedding gathers for orders 2..3
                for o in range(o_lo, 3):
                    L = profile[g][o]
                    for s in range(L):
                        col = g * MW + COLBASE[o] + s
                        if slots:
                            slab = slabs[o][g % 5]
                            dst = slab[:, s * EMB:(s + 1) * EMB]
                            op = mybir.AluOpType.bypass
                        elif rot:
                            dst = rot_tiles[rix[0] % rot][:]
                            rix[0] += 1
                            op = mybir.AluOpType.bypass
                        else:
                            dst = accs[(g, o)][:]
                            op = (mybir.AluOpType.bypass
                                  if (s == 0 or cce_off)
                                  else mybir.AluOpType.add)
                        spray(nc.gpsimd.indirect_dma_start(
                            out=dst,
                            out_offset=None,
                            in_=d_tabs[o][:],
                            in_offset=bass.IndirectOffsetOnAxis(
                                ap=t_meta[:, col:col + 1], axis=0),
                            compute_op=op))
                    next_q()

                if g > 1:
                    emit_tail(g - 2)

                if offload1:
                    # order-1 via one-hot histogram + PE matmul
                    s1 = profile[g][0]
                    H = t_H[g % 2]
                    # ids (int32 meta cols) -> f16, exact for ids <= 2048
                    nc.vector.tensor_copy(
                        out=t_ids1f[:, g * 12:g * 12 + s1],
                        in_=t_meta[:, g * MW:g * MW + s1])
                    for s in range(s1):
                        idb = bass.AP(t_ids1f[:].tensor, g * 12 + s,
                                      [t_ids1f[:].ap[0], [0, W1]])
                        if s == 0:
                            nc.vector.tensor_tensor(
                                out=H[:], in0=t_iota[:], in1=idb,
                                op=mybir.AluOpType.is_equal)
                        else:
                            oh = t_oh[g % 2]
                            nc.vector.tensor_tensor(
                                out=oh[:], in0=t_iota[:], in1=idb,
                                op=mybir.AluOpType.is_equal)
                            nc.vector.tensor_tensor(
                                out=H[:], in0=H[:], in1=oh[:],
                                op=mybir.AluOpType.add)
                    HT = t_HT[g % 2]
                    for k in range(NB1):
                        pT = ps_T[k % 2]
                        nc.tensor.transpose(
                            pT[:], H[:, k * P:(k + 1) * P], t_id[:])
                        nc.vector.tensor_copy(
                            out=HT[:, k * P:(k + 1) * P], in_=pT[:])
                    pE = ps_E[g % 4]
                    for k in range(NB1):
                        nc.tensor.matmul(
                            pE[:],
                            lhsT=HT[:, k * P:(k + 1) * P],
                            rhs=t_tab1[:, k * EMB:(k + 1) * EMB],
                            start=(k == 0), stop=(k == NB1 - 1))
                    in1 = bass.AP(t_rcp[:].tensor, g * 3 + 0,
                                  [t_rcp[:].ap[0], [0, EMB]])
                    nc.vector.tensor_tensor(
                        out=t_out[:, g * 768:g * 768 + EMB],
                        in0=pE[:], in1=in1, op=mybir.AluOpType.mult)

            if G >= 2:
                emit_tail(G - 2)
            emit_tail(G - 1)

    return nc


_NC_CACHE = {}


def _get_nc(profile, nq=NQ, unroll=1, offload1=OFFLOAD1,
            meta_host=META_HOST, **kw):
    key = (profile, nq, unroll, offload1, meta_host, tuple(sorted(kw.items())))
    if key not in _NC_CACHE:
        nc = _build(profile, num_swdge_queues=nq, unroll=unroll,
                    offload1=offload1, meta_host=meta_host, **kw)
        nc.finalize()
        _NC_CACHE[key] = nc
    return _NC_CACHE[key]


def _prep(inputs):
    tab0 = np.asarray(inputs['tab0'], np.float32)
    tabs = [np.asarray(inputs[f'tab{o+1}'], np.float32) for o in range(3)]
    ids = [np.asarray(inputs[f'ids{o+1}'], np.int64) for o in range(3)]
    cnt = [np.asarray(inputs[f'cnt{o+1}'], np.int64) for o in range(3)]

    meta = np.zeros((V, MW), np.int32)
    for o in range(3):
        meta[:, COLBASE[o]:COLBASE[o] + LS[o]] = ids[o]
        meta[:, CNTCOL + o] = cnt[o]
    # specials: slot 0 -> appended per-special row, others 0, cnt 1
    meta[:4, :CNTCOL] = 0
    meta[:4, CNTCOL:CNTCOL + 3] = 1
    nrows = (1001, 10001, 50001)
    for o in range(3):
        meta[:4, COLBASE[o]] = nrows[o] + np.arange(4)

    shared = {'metaI': meta}
    for o in range(3):
        tz = np.zeros((nrows[o] + 4, EMB), BF16)
        tz[1:nrows[o]] = tabs[o][1:].astype(BF16)
        tz[nrows[o]:] = tab0[:, o * EMB:(o + 1) * EMB].astype(BF16)
        shared[f'tab{o+1}z'] = tz
    shared['iotaH'] = np.tile(np.arange(W1, dtype=np.float16)[None, :],
                              (P, 1))
    tab1p = np.zeros((W1, EMB), BF16)
    tab1p[:1005] = shared['tab1z']
    shared['tab1p'] = tab1p

    # ---- dedup words, sort by length (cnt1 desc), deal to cores
    x = np.asarray(inputs['x'], np.int64).reshape(-1)
    ux, inv = np.unique(x, return_inverse=True)
    key = meta[ux, CNTCOL]                     # cnt1 = word length
    order_u = np.argsort(-key, kind='stable')  # descending
    su = ux[order_u]                           # sorted unique words
    n_u = len(su)
    n_pad = -(-n_u // (NCORES * P)) * (NCORES * P)
    su = np.concatenate([su, np.zeros(n_pad - n_u, np.int64)])
    Gc = n_pad // (NCORES * P)                 # groups per core

    core_words = [su[c::NCORES] for c in range(NCORES)]

    profile = []
    for g in range(Gc):
        mx = [1, 1, 1]
        for c in range(NCORES):
            seg = core_words[c][g * P:(g + 1) * P]
            for o in range(3):
                mx[o] = max(mx[o], int(meta[seg, CNTCOL + o].max()))
        profile.append(tuple(mx))
    profile = tuple(profile)

    # token t -> rank of its word in su -> (core r%8, row r//8)
    rank_of = np.empty(n_u, np.int64)
    rank_of[order_u] = np.arange(n_u)
    tok_rank = rank_of[inv]

    in_maps = []
    for c in range(NCORES):
        m = dict(shared)
        m['xpg'] = np.ascontiguousarray(
            core_words[c].reshape(Gc, P).T).astype(np.int32)
        m['metaT'] = np.ascontiguousarray(
            meta[core_words[c]].reshape(Gc, P, MW)
            .transpose(1, 0, 2).reshape(P, Gc * MW))
        in_maps.append(m)
    return in_maps, profile, tok_rank

def _run(nc, in_maps, trace=False):
    return run_bass_kernel_spmd(nc, in_maps, core_ids=list(range(NCORES)),
                                trace=trace)


def kernel(**inputs):
    in_maps, profile, tok_rank = _prep(inputs)
    nc = _get_nc(profile)
    res = _run(nc, in_maps)
    by_rank = np.stack([np.asarray(res.results[c]['out'])
                        for c in range(NCORES)])      # [core, row, 768]
    out = by_rank[tok_rank % NCORES, tok_rank // NCORES]
    return out.reshape(B, S, 768)


# revision 27
# speedup vs baseline: 1.0805x; 1.0246x over previous
"""N-gram embedding lookup kernel for Trainium2 (8 NeuronCores, Bass/Tile).

Problem: for each token x[b,s] (vocab 50000), gather precomputed n-gram
hash ids for orders 1..3 (12+11+10 slots), gather embedding rows from
three tables (1001/10001/50001 x 256 fp32), masked-mean each order,
concat to 768 dims; tokens x<4 take tab0[x] instead.

Environment constraints (verified on HW this session):
 - no HIPI ucode => custom bulk-gather (InstDMAGatherAnt) is unavailable
   (NRT_EXEC_UNIT_UNRECOVERABLE when executed);
 - walrus-native indirect DMA (InstDMACopy + dynamic AP) gathers exactly
   ONE row per partition per instruction (extra offset-AP indices are
   ignored; the descriptor reads out-row-size contiguous bytes from the
   single per-partition index), and each instruction serializes ~1.7us
   (bypass) / ~2.2us (CCE-add) of Pool-engine SWDGE descriptor
   generation. That serial gen time dominates; num_swdge_queues,
   acc dtype, and chain splitting measurably do NOT change it, so the
   only lever is issuing FEWER indirect-DMA instructions.

Design (measured ~325us/core vs ~970us for the naive 476-instruction
data-parallel layout; rel err 1.66e-3):
 - host: dedup x to unique words, sort by word length (cnt1 desc), deal
   round-robin to the 8 cores => every 128-token group holds words of
   nearly equal length, and per-group slot counts (the "profile")
   shrink from (12,11,10) to the group's actual max cnt per order. The
   Bass program is compiled per profile (cached) and outputs are
   scattered back to token positions on host.
 - host: per-token meta rows (ids/cnts, int32 [P, G*40]) are shipped
   directly; specials (x<4) are folded into the tables as 4 appended
   rows so no separate patch pass exists.
 - chip, order 1 (1001-row table): offloaded off the Pool engine
   entirely - DVE builds per-token bucket histograms via f16 iota
   is_equal one-hots, PE transposes them and matmuls against the
   SBUF-resident padded table, accumulating in PSUM.
 - chip, orders 2/3: profile[g][o] independent bypass indirect-DMA
   gathers (bf16 rows -> f32 cast in the DMA) into slot slabs, then one
   DVE tensor_reduce + 1/cnt scale per (group, order). Slabs rotate
   5-deep; each group's reduce is issued two groups late so the Pool
   engine never waits on DVE (measured -18us vs one group late).
 - per-group stores overlap the Pool gather stream on the SP engine.
"""
import numpy as np
import ml_dtypes
from contextlib import ExitStack

from concourse import bacc, bass, mybir, tile
from concourse.bass_utils import run_bass_kernel_spmd

BF16 = ml_dtypes.bfloat16

NCORES = 8
B, S = 8, 2048
TOK = B * S
TPC = TOK // NCORES          # 2048 tokens per core
P = 128
G = TPC // P                 # 16 groups
EMB = 256
V = 50000
LS = (12, 11, 10)
COLBASE = (0, 12, 23)        # meta col of slot 0 per order
CNTCOL = 33                  # meta cols 33..35 = cnt1..3
MW = 40                      # meta row width (int32)
NQ = 1                       # SWDGE queues to spray across
OFFLOAD1 = True              # compute order-1 via DVE one-hots + PE matmul
META_HOST = True             # ship per-token meta rows from host (no gather)
SLOTS = True                 # bypass gathers into slot slabs + DVE reduce
W1 = 1024                    # order-1 bucket space (1005 used, padded)
NB1 = W1 // 128


def _build(profile, num_swdge_queues=NQ, unroll=1, offload1=OFFLOAD1,
           meta_host=META_HOST, cce_off=False, acc_bf16=False, rot=0,
           slots=SLOTS):
    G = len(profile)
    TPC = G * P
    i32, f32, bf16 = mybir.dt.int32, mybir.dt.float32, mybir.dt.bfloat16
    f16 = mybir.dt.float16
    nc = bacc.Bacc("TRN2", target_bir_lowering=False, debug=False,
                   num_devices=NCORES, num_swdge_queues=num_swdge_queues)

    if meta_host:
        d_metaT = nc.dram_tensor("metaT", [P, G * MW], i32,
                                 kind="ExternalInput")
    else:
        d_meta = nc.dram_tensor("metaI", [V, MW], i32, kind="ExternalInput")
        d_xpg = nc.dram_tensor("xpg", [P, G], i32, kind="ExternalInput")
    d_tabs = [nc.dram_tensor(f"tab{o+1}z", [(1005, 10005, 50005)[o], EMB],
                             bf16, kind="ExternalInput") for o in range(3)]
    d_out = nc.dram_tensor("out", [TPC, 768], f32, kind="ExternalOutput")
    if offload1:
        d_iota = nc.dram_tensor("iotaH", [P, W1], f16, kind="ExternalInput")
        d_tab1p = nc.dram_tensor("tab1p", [W1, EMB], bf16,
                                 kind="ExternalInput")

    swdge_q = [0]

    def spray(inst):
        # round-robin independent DMAs across SWDGE queues; keep each
        # accumulation chain on one queue (callers rotate per chain)
        if num_swdge_queues > 1:
            q = swdge_q[0] % num_swdge_queues
            if q:
                inst.ins.queue = f"qPoolDynamic{q}"
        return inst

    def next_q():
        swdge_q[0] += 1

    with ExitStack() as ctx:
        tc = ctx.enter_context(tile.TileContext(nc))
        pool = ctx.enter_context(tc.tile_pool(name="sbuf", bufs=1))

        t_xpg = pool.tile([P, G], i32)
        t_meta = pool.tile([P, G * MW], i32)
        o_lo = 1 if offload1 else 0
        acc_dt = bf16 if acc_bf16 else f32
        rot_tiles = [pool.tile([P, EMB], acc_dt, name=f"rot_{i}")
                     for i in range(rot)] if rot else None
        if slots:
            lmax = {o: max(p[o] for p in profile) for o in range(o_lo, 3)}
            slabs = {o: [pool.tile([P, lmax[o] * EMB], f32,
                                   name=f"slab_{o}_{i}") for i in range(5)]
                     for o in range(o_lo, 3)}
            accs = None
        else:
            accs = {(g, o): (rot_tiles[(g * 3 + o) % rot] if rot else
                             pool.tile([P, EMB], acc_dt, name=f"acc_{g}_{o}"))
                    for g in range(G) for o in range(o_lo, 3)}
        t_cntf = pool.tile([P, G * 3], f32)
        t_rcp = pool.tile([P, G * 3], f32)
        t_out = pool.tile([P, G * 768], f32)
        if offload1:
            from concourse.masks import make_identity
            psum = ctx.enter_context(
                tc.tile_pool(name="psum", bufs=1, space="PSUM"))
            t_iota = pool.tile([P, W1], f16)
            t_tab1 = pool.tile([P, NB1 * EMB], bf16)
            t_id = pool.tile([P, P], f16)
            t_ids1f = pool.tile([P, G * 12], f16)
            t_H = [pool.tile([P, W1], f16, name=f"H{i}") for i in range(2)]
            t_oh = [pool.tile([P, W1], f16, name=f"oh{i}") for i in range(2)]
            t_HT = [pool.tile([P, NB1 * P], bf16, name=f"HT{i}")
                    for i in range(2)]
            ps_T = [psum.tile([P, P], f16, name=f"psT{i}") for i in range(2)]
            ps_E = [psum.tile([P, EMB], f32, name=f"psE{i}")
                    for i in range(4)]
            nc.sync.dma_start(out=t_iota[:], in_=d_iota[:])
            nc.sync.dma_start(
                out=bass.AP(t_tab1[:].tensor, 0,
                            [t_tab1[:].ap[0], [EMB, NB1], [1, EMB]]),
                in_=bass.AP(d_tab1p, 0, [[EMB, P], [P * EMB, NB1], [1, EMB]]))
            make_identity(nc, t_id[:])

        if unroll > 1:
            # hardware loop for benchmarking: body is idempotent
            ctx.enter_context(tc.For_i(0, unroll))

        if True:
            if meta_host:
                nc.sync.dma_start(out=t_meta[:], in_=d_metaT[:])
            else:
                nc.sync.dma_start(out=t_xpg[:], in_=d_xpg[:])
                # ---- meta gathers: one per group, row [40 int32] per token
                for g in range(G):
                    spray(nc.gpsimd.indirect_dma_start(
                        out=t_meta[:, g * MW:(g + 1) * MW],
                        out_offset=None,
                        in_=d_meta[:],
                        in_offset=bass.IndirectOffsetOnAxis(
                            ap=t_xpg[:, g:g + 1], axis=0)))
                    next_q()

            # ---- 1/cnt
            nc.vector.tensor_copy(
                out=t_cntf[:],
                in_=bass.AP(t_meta[:].tensor, CNTCOL,
                            [t_meta[:].ap[0], [MW, G], [1, 3]]))
            nc.vector.reciprocal(out=t_rcp[:], in_=t_cntf[:])

            # ---- per-group pipeline: Pool gathers / DVE+PE order-1 /
            #      DVE reduces+scales / SP store.  Issue order per group
            #      keeps slab/H/psum rotation windows correct and lets
            #      every engine run concurrently.
            rix = [0]

            def emit_tail(g):
                # DVE: reduce slots (v4) / scale into output tile
                for o in range(o_lo, 3):
                    out_ap = t_out[:, g * 768 + o * 256:
                                   g * 768 + (o + 1) * 256]
                    in1 = bass.AP(t_rcp[:].tensor, g * 3 + o,
                                  [t_rcp[:].ap[0], [0, 256]])
                    if slots:
                        L = profile[g][o]
                        slab = slabs[o][g % 5]
                        if L == 1:
                            nc.vector.tensor_tensor(
                                out=out_ap, in0=slab[:, 0:EMB], in1=in1,
                                op=mybir.AluOpType.mult)
                        else:
                            nc.vector.tensor_reduce(
                                out=out_ap,
                                in_=bass.AP(slab[:].tensor, 0,
                                            [slab[:].ap[0], [1, EMB],
                                             [EMB, L]]),
                                axis=mybir.AxisListType.X,
                                op=mybir.AluOpType.add, opt_input=False)
                            nc.vector.tensor_tensor(out=out_ap, in0=out_ap,
                                                    in1=in1,
                                                    op=mybir.AluOpType.mult)
                    else:
                        acc = accs[(g, o)]
                        nc.vector.tensor_tensor(out=out_ap, in0=acc[:],
                                                in1=in1,
                                                op=mybir.AluOpType.mult)
                # store: SBUF (p, g, 768) -> DRAM row g*128+p
                nc.sync.dma_start(
                    out=bass.AP(d_out, g * P * 768, [[768, P], [1, 768]]),
                    in_=t_out[:, g * 768:(g + 1) * 768])

            for g in range(G):
                # Pool: embedding gathers for orders 2..3
                for o in range(o_lo, 3):
                    L = profile[g][o]
                    for s in range(L):
                        col = g * MW + COLBASE[o] + s
                        if slots:
                            slab = slabs[o][g % 5]
                            dst = slab[:, s * EMB:(s + 1) * EMB]
                            op = mybir.AluOpType.bypass
                        elif rot:
                            dst = rot_tiles[rix[0] % rot][:]
                            rix[0] += 1
                            op = mybir.AluOpType.bypass
                        else:
                            dst = accs[(g, o)][:]
                            op = (mybir.AluOpType.bypass
                                  if (s == 0 or cce_off)
                                  else mybir.AluOpType.add)
                        spray(nc.gpsimd.indirect_dma_start(
                            out=dst,
                            out_offset=None,
                            in_=d_tabs[o][:],
                            in_offset=bass.IndirectOffsetOnAxis(
                                ap=t_meta[:, col:col + 1], axis=0),
                            compute_op=op))
                    next_q()

                if g > 1:
                    emit_tail(g - 2)
                if g == G - 1 and g >= 1:
                    emit_tail(g - 1)

                if offload1:
                    # order-1 via one-hot histogram + PE matmul
                    s1 = profile[g][0]
                    H = t_H[g % 2]
                    # ids (int32 meta cols) -> f16, exact for ids <= 2048
                    nc.vector.tensor_copy(
                        out=t_ids1f[:, g * 12:g * 12 + s1],
                        in_=t_meta[:, g * MW:g * MW + s1])
                    for s in range(s1):
                        idb = bass.AP(t_ids1f[:].tensor, g * 12 + s,
                                      [t_ids1f[:].ap[0], [0, W1]])
                        if s == 0:
                            nc.vector.tensor_tensor(
                                out=H[:], in0=t_iota[:], in1=idb,
                                op=mybir.AluOpType.is_equal)
                        else:
                            oh = t_oh[g % 2]
                            nc.vector.tensor_tensor(
                                out=oh[:], in0=t_iota[:], in1=idb,
                                op=mybir.AluOpType.is_equal)
                            nc.vector.tensor_tensor(
                                out=H[:], in0=H[:], in1=oh[:],
                                op=mybir.AluOpType.add)
                    HT = t_HT[g % 2]
                    for k in range(NB1):
                        pT = ps_T[k % 2]
                        nc.tensor.transpose(
                            pT[:], H[:, k * P:(k + 1) * P], t_id[:])
                        nc.vector.tensor_copy(
                            out=HT[:, k * P:(k + 1) * P], in_=pT[:])
                    pE = ps_E[g % 4]
                    for k in range(NB1):
                        nc.tensor.matmul(
                            pE[:],
                            lhsT=HT[:, k * P:(k + 1) * P],
                            rhs=t_tab1[:, k * EMB:(k + 1) * EMB],
                            start=(k == 0), stop=(k == NB1 - 1))
                    in1 = bass.AP(t_rcp[:].tensor, g * 3 + 0,
                                  [t_rcp[:].ap[0], [0, EMB]])
                    nc.vector.tensor_tensor(
                        out=t_out[:, g * 768:g * 768 + EMB],
                        in0=pE[:], in1=in1, op=mybir.AluOpType.mult)

            emit_tail(G - 1)

    return nc


_NC_CACHE = {}


def _get_nc(profile, nq=NQ, unroll=1, offload1=OFFLOAD1,
            meta_host=META_HOST, **kw):
    key = (profile, nq, unroll, offload1, meta_host, tuple(sorted(kw.items())))
    if key not in _NC_CACHE:
        nc = _build(profile, num_swdge_queues=nq, unroll=unroll,
                    offload1=offload1, meta_host=meta_host, **kw)
        nc.finalize()
        _NC_CACHE[key] = nc
    return _NC_CACHE[key]


def _prep(inputs):
    tab0 = np.asarray(inputs['tab0'], np.float32)
    tabs = [np.asarray(inputs[f'tab{o+1}'], np.float32) for o in range(3)]
    ids = [np.asarray(inputs[f'ids{o+1}'], np.int64) for o in range(3)]
    cnt = [np.asarray(inputs[f'cnt{o+1}'], np.int64) for o in range(3)]

    meta = np.zeros((V, MW), np.int32)
    for o in range(3):
        meta[:, COLBASE[o]:COLBASE[o] + LS[o]] = ids[o]
        meta[:, CNTCOL + o] = cnt[o]
    # specials: slot 0 -> appended per-special row, others 0, cnt 1
    meta[:4, :CNTCOL] = 0
    meta[:4, CNTCOL:CNTCOL + 3] = 1
    nrows = (1001, 10001, 50001)
    for o in range(3):
        meta[:4, COLBASE[o]] = nrows[o] + np.arange(4)

    shared = {'metaI': meta}
    for o in range(3):
        tz = np.zeros((nrows[o] + 4, EMB), BF16)
        tz[1:nrows[o]] = tabs[o][1:].astype(BF16)
        tz[nrows[o]:] = tab0[:, o * EMB:(o + 1) * EMB].astype(BF16)
        shared[f'tab{o+1}z'] = tz
    shared['iotaH'] = np.tile(np.arange(W1, dtype=np.float16)[None, :],
                              (P, 1))
    tab1p = np.zeros((W1, EMB), BF16)
    tab1p[:1005] = shared['tab1z']
    shared['tab1p'] = tab1p

    # ---- dedup words, sort by length (cnt1 desc), deal to cores
    x = np.asarray(inputs['x'], np.int64).reshape(-1)
    ux, inv = np.unique(x, return_inverse=True)
    key = meta[ux, CNTCOL]                     # cnt1 = word length
    order_u = np.argsort(-key, kind='stable')  # descending
    su = ux[order_u]                           # sorted unique words
    n_u = len(su)
    n_pad = -(-n_u // (NCORES * P)) * (NCORES * P)
    su = np.concatenate([su, np.zeros(n_pad - n_u, np.int64)])
    Gc = n_pad // (NCORES * P)                 # groups per core

    core_words = [su[c::NCORES] for c in range(NCORES)]

    profile = []
    for g in range(Gc):
        mx = [1, 1, 1]
        for c in range(NCORES):
            seg = core_words[c][g * P:(g + 1) * P]
            for o in range(3):
                mx[o] = max(mx[o], int(meta[seg, CNTCOL + o].max()))
        profile.append(tuple(mx))
    profile = tuple(profile)

    # token t -> rank of its word in su -> (core r%8, row r//8)
    rank_of = np.empty(n_u, np.int64)
    rank_of[order_u] = np.arange(n_u)
    tok_rank = rank_of[inv]

    in_maps = []
    for c in range(NCORES):
        m = dict(shared)
        m['xpg'] = np.ascontiguousarray(
            core_words[c].reshape(Gc, P).T).astype(np.int32)
        m['metaT'] = np.ascontiguousarray(
            meta[core_words[c]].reshape(Gc, P, MW)
            .transpose(1, 0, 2).reshape(P, Gc * MW))
        in_maps.append(m)
    return in_maps, profile, tok_rank

def _run(nc, in_maps, trace=False):
    return run_bass_kernel_spmd(nc, in_maps, core_ids=list(range(NCORES)),
                                trace=trace)


def kernel(**inputs):
    in_maps, profile, tok_rank = _prep(inputs)
    nc = _get_nc(profile)
    res = _run(nc, in_maps)
    by_rank = np.stack([np.asarray(res.results[c]['out'])
                        for c in range(NCORES)])      # [core, row, 768]
    out = by_rank[tok_rank % NCORES, tok_rank // NCORES]
    return out.reshape(B, S, 768)
